# revision 3
# baseline (speedup 1.0000x reference)
# Masked-reset LSTM (MemoryEmbedding) Trainium2 kernel.
#
# Strategy: the episode-reset mask M (binary) splits every batch row into
# independent episodes, each starting from a known state (S[row] at t=0,
# zeros at every reset).  On the host we segment all B*T steps into
# episodes, sort them by length (desc) and deal them round-robin onto the
# 8 cores.  The device then runs a "wavefront" over relative step r
# (round r processes step r of every episode longer than r); because the
# episodes are sorted, the active set at every round is a prefix of the
# slot axis, so all device work is dense.  Sequential depth drops from
# T=2048 to max-episode-length (~56 for 10% reset rate).
#
# Each core's episodes are split into two interleaved groups with
# independent state; the two dependency chains ping-pong on the engines,
# hiding the per-round semaphore/issue latency.
#
# Device layout is fully transposed (gate/hidden units on partitions,
# episode slots on the free axis):
#   z^T[j*128:(j+1)*128, slot] = W1[:, j]^T @ obs1^T + R2[:, j]^T @ h^T
# with W1 = [W_embed @ kernel ; b_embed @ kernel + bias ; R2^T h0 rows]
# and gate columns permuted to [i|f|o|g] so one sigmoid covers blocks
# 0..5 and one tanh covers blocks 6..7.
#
# Initial states are folded into the weights: each core has <= 32
# episodes inheriting S[row] (one per batch row); row 65+i of W1 holds
# R2^T @ h0_i and the episode's round-0 obs column carries a one-hot at
# that row.  Round 0 therefore runs NO h-side matmuls at all (h0 == 0
# after folding), and there is no initial-h input tensor.
#
# z is accumulated in PSUM tiles of [128, 8, 256] f32 (gate stride 256
# floats keeps every per-gate matmul output slice inside one 2KB PSUM
# bank for any piece width <= 128 at offsets 0/128), so schedule widths
# are exact episode counts - no padding - and one sigmoid/tanh pair
# covers up to 256 slots.
#
# All element-wise work runs on DVE in bf16 (2x mode); h is written into
# a persistent SBUF buffer that doubles as next round's matmul input and
# is DMA'd to HBM in multi-round chunks from SP.

import json

import ml_dtypes
import numpy as np

OBS_DIM, EMBED_DIM, HIDDEN = 64, 128, 256
BATCH, SEQLEN = 32, 2048
N_CORES = 8
N_GROUPS = 2
GH = HIDDEN // 128  # 2 gate-partition tiles per gate
KDIM = 128  # contraction rows: 64 obs + 1 bias + <=32 h0-fold + pad
ZW = 256  # z-tile width (one sigmoid/tanh pair per tile)
CHUNK = 8  # rounds per output-DMA flush

BF16 = ml_dtypes.bfloat16

# ---------------------------------------------------------------------------
# host-side helpers
# ---------------------------------------------------------------------------


def _episodes(M2):
    """M2: [B, T] binary mask -> arrays (row, start, length), unsorted."""
    rows, starts, lens = [], [], []
    for b in range(M2.shape[0]):
        bounds = np.flatnonzero(M2[b] > 0.5)
        bs = np.unique(np.concatenate([[0], bounds, [M2.shape[1]]]))
        ls = np.diff(bs)
        keep = ls > 0
        rows.append(np.full(keep.sum(), b))
        starts.append(bs[:-1][keep])
        lens.append(ls[keep])
    return (np.concatenate(rows), np.concatenate(starts), np.concatenate(lens))


def _pieces(k):
    """Split k slots into <=128-wide matmul pieces, exact (no padding)."""
    ws = [128] * (k // 128)
    if k % 128:
        ws.append(k % 128)
    return ws


def _ztiles(k):
    """Split k slots into <=256-wide z tiles, exact."""
    ws = [ZW] * (k // ZW)
    if k % ZW:
        ws.append(k % ZW)
    return ws


# ---------------------------------------------------------------------------
# bass program
# ---------------------------------------------------------------------------

# This neuronxcc build caps sync commands (waits + updates) per instruction
# by ISA struct size: plain ops like Drain (CTRL_NO_STRUCT) get 1 slot,
# larger structs (matmul, tensor ops, EventSemaphore, ...) get 2.
_SMALL_OPS = {"Drain", "NoOp", "Nop"}


def _patch_bir_json(raw: bytes) -> bytes:
    """Hoist excess semaphore waits onto carrier EventSemaphore instructions
    inserted immediately before the over-cap instruction on the same engine —
    engines execute in order and Tile semaphores are monotonic, so this
    preserves semantics.  EventSemaphore is a pure sequencer op (~23 ns) that
    does not flush the engine pipeline the way Drain does."""
    m = json.loads(raw)
    n = [0]
    for f in m["functions"]:
        for bb in f["blocks"]:
            out = []
            for inst in bb["instructions"]:
                si = inst.get("sync_info")
                waits = (si or {}).get("on_wait") or []
                upds = (si or {}).get("on_update") or []
                cap = 1 if inst.get("opcode") in _SMALL_OPS else 2
                keep = max(cap - len(upds), 0)
                if si and len(waits) > keep:
                    eng = inst.get("engine", "SP")
                    # EventSemaphore is ~23-52ns on most engines but ~2.6us
                    # on the Pool (GpSimd) DSP; a Pool Drain is ~51ns.
                    cop, step = ("Drain", 1) if eng == "Pool" else ("EventSemaphore", 2)
                    extra = waits[: len(waits) - keep]
                    si["on_wait"] = waits[len(waits) - keep :]
                    for k in range(0, len(extra), step):
                        n[0] += 1
                        out.append(
                            {
                                "name": f"I-syncw-{n[0]}",
                                "opcode": cop,
                                "engine": eng,
                                "ins": [],
                                "outs": [],
                                "debug": inst.get("debug", 0),
                                "sync_info": {
                                    "on_wait": extra[k : k + step],
                                    "on_update": [],
                                },
                            }
                        )
                out.append(inst)
            bb["instructions"] = out
    return json.dumps(m).encode()


def _build_bass(R, scheds, npad, s0caps, gblocks, obs_splits):
    """scheds: per group dict with widths[r] (slot count, 0 = inactive),
    offs[r].  gblocks: per group (start, end) column range (g-major).
    obs_splits: per group column where the obs DMA is split in two."""
    import concourse.bass as bass
    import concourse.mybir as mybir
    import concourse.tile as tile

    f32 = mybir.dt.float32
    bf16 = mybir.dt.bfloat16
    SIG = mybir.ActivationFunctionType.Sigmoid
    TANH = mybir.ActivationFunctionType.Tanh

    s0tot = sum(s0caps)
    nc = bass.Bass()
    obsT = nc.dram_tensor("obsT", [KDIM, npad], bf16, kind="ExternalInput")
    W1 = nc.dram_tensor("W1", [KDIM, 4 * HIDDEN], bf16, kind="ExternalInput")
    R2a = nc.dram_tensor("R2a", [128, 4 * HIDDEN], bf16, kind="ExternalInput")
    R2b = nc.dram_tensor("R2b", [128, 4 * HIDDEN], bf16, kind="ExternalInput")
    ic = nc.dram_tensor("ic", [128, GH, s0tot], bf16, kind="ExternalInput")
    HT = nc.dram_tensor("HT", [128, GH, npad], bf16, kind="ExternalOutput")

    with tile.TileContext(nc) as tc:
        with (
            tc.tile_pool(name="const", bufs=1) as cpool,
            tc.tile_pool(name="state", bufs=1) as spool,
            tc.tile_pool(name="acts", bufs=3) as apool,
            tc.tile_pool(name="zp", bufs=2, space="PSUM") as zpool,
        ):
            w1s = cpool.tile([KDIM, 4 * HIDDEN], bf16)
            nc.sync.dma_start(out=w1s, in_=W1[:, :])
            r2a = cpool.tile([128, 4 * HIDDEN], bf16)
            nc.sync.dma_start(out=r2a, in_=R2a[:, :])
            r2b = cpool.tile([128, 4 * HIDDEN], bf16)
            nc.sync.dma_start(out=r2b, in_=R2b[:, :])

            # persistent h buffer: written once per column, read by the next
            # round's matmuls, exported to HBM in multi-round chunks.
            ht_sb = cpool.tile([128, GH, npad], bf16)

            obs = cpool.tile([KDIM, npad], bf16)

            c_st = []
            goff = 0
            for g in range(N_GROUPS):
                cs = spool.tile(
                    [128, GH, s0caps[g]], bf16, name=f"c_st{g}", tag=f"c{g}"
                )
                nc.sync.dma_start(out=cs, in_=ic[:, :, goff : goff + s0caps[g]])
                c_st.append(cs)
                goff += s0caps[g]

            # obs chunks: early rounds of every group first, remainders after
            for g in range(N_GROUPS):
                a, _ = gblocks[g]
                nc.sync.dma_start(out=obs[:, a : obs_splits[g]], in_=obsT[:, a : obs_splits[g]])
            for g in range(N_GROUPS):
                _, b = gblocks[g]
                if obs_splits[g] < b:
                    nc.sync.dma_start(out=obs[:, obs_splits[g] : b], in_=obsT[:, obs_splits[g] : b])

            # h source for round r of group g: none for r==0 (folded into
            # W1), else the ht_sb slice written by round r-1 (prefix
            # property: k_r <= k_{r-1}).
            h_src = [None] * N_GROUPS
            pend = [None] * N_GROUPS  # round awaiting p3, per group
            dma_pend = [None] * N_GROUPS  # start col of unflushed output

            def active(g, r):
                return r < len(scheds[g]["widths"]) and scheds[g]["widths"][r] > 0

            def p1(g, r):
                kp = scheds[g]["widths"][r]
                off = scheds[g]["offs"][r]
                src = h_src[g]
                sig = apool.tile([128, 6, kp], bf16, name=f"sig{g}", tag=f"sig{g}")
                tg = apool.tile([128, GH, kp], bf16, name=f"tg{g}", tag=f"tg{g}")
                col = 0
                for wt in _ztiles(kp):
                    z = zpool.tile([128, 8, ZW], f32, name=f"z{g}", tag="z")
                    for j in range(8):
                        c0 = 0
                        for w in _pieces(wt):
                            zj = z[:, j, c0 : c0 + w]
                            ob = obs[:, off + col + c0 : off + col + c0 + w]
                            if src is None:
                                nc.tensor.matmul(
                                    zj,
                                    w1s[:, j * 128 : (j + 1) * 128],
                                    ob,
                                    start=True,
                                    stop=True,
                                )
                            else:
                                a0 = src + col + c0
                                nc.tensor.matmul(
                                    zj,
                                    w1s[:, j * 128 : (j + 1) * 128],
                                    ob,
                                    start=True,
                                    stop=False,
                                )
                                nc.tensor.matmul(
                                    zj,
                                    r2a[:, j * 128 : (j + 1) * 128],
                                    ht_sb[:, 0, a0 : a0 + w],
                                    start=False,
                                    stop=False,
                                )
                                nc.tensor.matmul(
                                    zj,
                                    r2b[:, j * 128 : (j + 1) * 128],
                                    ht_sb[:, 1, a0 : a0 + w],
                                    start=False,
                                    stop=True,
                                )
                            c0 += w
                    nc.scalar.activation(
                        sig[:, :, col : col + wt], z[:, 0:6, 0:wt], SIG
                    )
                    nc.scalar.activation(
                        tg[:, :, col : col + wt], z[:, 6:8, 0:wt], TANH
                    )
                    col += wt
                h_src[g] = off
                return sig, tg, kp, off

            def p2(g, st):
                sig, tg, kp, off = st
                cs = c_st[g]
                ig = apool.tile([128, GH, kp], bf16, name=f"ig{g}", tag=f"ig{g}")
                nc.vector.tensor_mul(ig, sig[:, 0:2, :], tg)
                fc = apool.tile([128, GH, kp], bf16, name=f"fc{g}", tag=f"fc{g}")
                nc.vector.tensor_mul(fc, sig[:, 2:4, :], cs[:, :, 0:kp])
                nc.vector.tensor_add(cs[:, :, 0:kp], ig, fc)

            def p3(g, st, r):
                sig, tg, kp, off = st
                cs = c_st[g]
                tc_t = apool.tile([128, GH, kp], bf16, name=f"tc{g}", tag=f"tc{g}")
                nc.scalar.activation(tc_t, cs[:, :, 0:kp], TANH)
                nc.vector.tensor_mul(
                    ht_sb[:, :, off : off + kp], sig[:, 4:6, :], tc_t
                )
                if dma_pend[g] is None:
                    dma_pend[g] = off
                if (r + 1) % CHUNK == 0 or not active(g, r + 1):
                    a, b = dma_pend[g], off + kp
                    nc.sync.dma_start(out=HT[:, :, a:b], in_=ht_sb[:, :, a:b])
                    dma_pend[g] = None

            # ring order per round: [p1(g), p3(g-1, pending), p2(g)] for each
            # group g — every group's tail (p3) lands one slot after its p2,
            # and its next-round matmuls (p1) come well after its p3.
            for r in range(R):
                for g in range(N_GROUPS):
                    st = p1(g, r) if active(g, r) else None
                    gp = (g - 1) % N_GROUPS
                    if pend[gp] is not None:
                        p3(gp, pend[gp][0], pend[gp][1])
                        pend[gp] = None
                    if st is not None:
                        p2(g, st)
                        pend[g] = (st, r)
            for g in range(N_GROUPS):
                if pend[g] is not None:
                    p3(g, pend[g][0], pend[g][1])

    orig = nc.to_json_bytes
    nc.to_json_bytes = lambda: _patch_bir_json(orig())
    return nc


# ---------------------------------------------------------------------------
# entry point
# ---------------------------------------------------------------------------

LAST_RESULT = None  # BassKernelResults of the most recent run (for profiling)


def kernel(obs, S, M, W_embed, b_embed, kernel, rec_kernel, bias):
    import os

    from concourse.bass_utils import run_bass_kernel_spmd

    obs = np.asarray(obs, np.float32)
    S = np.asarray(S, np.float32)
    M = np.asarray(M, np.float32)
    W_embed = np.asarray(W_embed, np.float32)
    b_embed = np.asarray(b_embed, np.float32)
    kernel_w = np.asarray(kernel, np.float32)
    rec_kernel = np.asarray(rec_kernel, np.float32)
    bias = np.asarray(bias, np.float32)

    B = S.shape[0]
    T = obs.shape[0] // B
    H = HIDDEN

    # ---- weights: fold embedding, add bias row, permute gates to [i|f|o|g]
    perm = np.concatenate(
        [np.arange(0, 2 * H), np.arange(3 * H, 4 * H), np.arange(2 * H, 3 * H)]
    )
    w1 = np.zeros((KDIM, 4 * H), np.float32)
    w1[:OBS_DIM] = (W_embed @ kernel_w)[:, perm]
    w1[OBS_DIM] = (b_embed @ kernel_w + bias)[perm]
    r2 = rec_kernel[:, perm]

    # ---- episode segmentation
    M2 = M.reshape(B, T)
    rows, starts, lens = _episodes(M2)
    order = np.argsort(-lens, kind="stable")
    rows, starts, lens = rows[order], starts[order], lens[order]
    R = int(lens.max())

    # per (core, group) episode lists: deal the globally sorted episodes
    # round-robin so every (core, group) slot gets equally long episodes.
    eps = {}  # (c, g) -> (row, start, len), sorted desc by len
    for c in range(N_CORES):
        for g in range(N_GROUPS):
            k = N_GROUPS * c + g
            step = N_GROUPS * N_CORES
            eps[(c, g)] = (rows[k::step], starts[k::step], lens[k::step])

    # shared schedules (one compiled SPMD program): per group, per round,
    # the max active count over cores (exact widths, no padding).
    scheds = []
    off = 0
    offs_flat = {}
    gblocks = []
    obs_splits = []
    for g in range(N_GROUPS):
        widths, offs = [], []
        blk0 = off
        for r in range(R):
            k = max(int((eps[(c, g)][2] > r).sum()) for c in range(N_CORES))
            if k == 0:
                break
            widths.append(k)
            offs.append(off)
            offs_flat[(g, r)] = off
            off += k
        scheds.append({"widths": widths, "offs": offs})
        gblocks.append((blk0, off))
        obs_splits.append(offs[CHUNK] if len(offs) > CHUNK else off)
    npad = off
    s0caps = [scheds[g]["widths"][0] for g in range(N_GROUPS)]

    # ---- per-core packed inputs
    in_maps = []
    scat_src = []
    scat_col = []
    for c in range(N_CORES):
        obs1 = np.zeros((KDIM, npad), np.float32)
        obs1[OBS_DIM, :] = 1.0
        cols_all, srcs_all = [], []
        icb = np.zeros((128, GH, sum(s0caps)), np.float32)
        w1c = w1.copy()
        ns = 0  # next free W1 fold row (65 + ns)
        goff = 0
        for g in range(N_GROUPS):
            er, es, el = eps[(c, g)]
            for r in range(len(scheds[g]["widths"])):
                k = int((el > r).sum())
                if k == 0:
                    break
                off_r = offs_flat[(g, r)]
                cols_all.append(off_r + np.arange(k))
                srcs_all.append(er[:k] * T + es[:k] + r)
            idx = np.flatnonzero(es == 0)  # episodes inheriting S[row]
            for i in idx:
                h0 = S[er[i], :H]
                if np.any(h0):
                    w1c[OBS_DIM + 1 + ns] = h0 @ r2
                    obs1[OBS_DIM + 1 + ns, offs_flat[(g, 0)] + i] = 1.0
                    ns += 1
            for gg in range(GH):
                icb[:, gg, goff + idx] = S[
                    er[idx], H + gg * 128 : H + (gg + 1) * 128
                ].T
            goff += s0caps[g]
        cols_all = np.concatenate(cols_all)
        srcs_all = np.concatenate(srcs_all)
        obs1[:OBS_DIM, cols_all] = obs[srcs_all].T

        in_maps.append(
            {
                "obsT": obs1.astype(BF16),
                "W1": w1c.astype(BF16),
                "R2a": r2[:128].astype(BF16),
                "R2b": r2[128:].astype(BF16),
                "ic": icb.astype(BF16),
            }
        )
        scat_src.append(srcs_all)
        scat_col.append(cols_all)

    nc = _build_bass(R, scheds, npad, s0caps, gblocks, obs_splits)
    trace = bool(int(os.environ.get("KERNEL_TRACE", "0")))
    res = run_bass_kernel_spmd(
        nc, in_maps, core_ids=list(range(N_CORES)), trace=trace
    )
    global LAST_RESULT
    LAST_RESULT = res

    memory = np.zeros((B * T, H), np.float32)
    for c in range(N_CORES):
        ht = np.asarray(res.results[c]["HT"]).astype(np.float32)  # [128, GH, npad]
        memory[scat_src[c], :128] = ht[:, 0, scat_col[c]].T
        memory[scat_src[c], 128:] = ht[:, 1, scat_col[c]].T
    return memory
